# revision 1
# baseline (speedup 1.0000x reference)
"""Trainium2 Bass kernel for nn_CausalSelfAttention (B=4, L=2048, D=1024, H=16).

Sharding: batch x head-group.  Core c handles batch c//2 and heads
8*(c%2) .. 8*(c%2)+8  (8 heads = 4 pairs), i.e. 2 cores per batch, each
producing a partial projection yT_c = proj_w[rows_c].T @ O_c^T of shape
[D, T].  The host sums the 2 partials per batch, transposes, adds proj_b.

All matmul operands are bf16 (PSUM accumulation stays fp32): at N>=256
bf16 streams 1 cycle/row like fp32r, and on the causal-diagonal N=128
blocks it avoids fp32r's 4 cycles/row penalty.  x is transposed to
[D, T] on the host, so the device does ZERO x transposes; V is
transposed token-major via XBAR dma_start_transpose (14ns/16x128 tile),
so the PE only runs real GEMMs + the RoPE signed-permutation matmul.

Device pipeline per core (per 512-token chunk):
  xt tiles [d,tok] <- plain DMA from host-transposed xT
  qkvT [feat,tok] = w.T @ xt  (8 K-tiles, fp32 PSUM), bias folded into
      the PSUM->SBUF copy via DVE tensor_scalar_add (no K=1 bias matmul)
  RoPE: rot via psgn matmul, qT/kT = raw*cos + rot*sin (DVE, bf16 2x)
  V: dma_start_transpose to [tok, hd|1] tiles (65th col = ones -> PV
      matmul emits softmax row-sums for free)
  attention per head-pair: S^T = KT.T @ QT -> exp on ACT (scale=1/8,
      bf16 out) -> causal via affine_select on diagonal blocks ->
      O[65,512] += [V|1].T @ P^T
  normalize: 1/rowsum = exp(-ln(rowsum)) on ACT (same act table as the
      attention exp), gpsimd partition_broadcast, DVE mul -> ot bf16
  proj: yT += pw.T @ ot  (4 K-tiles per out tile)

Scheduling: engines execute their queues in order, and the PE p-state
only reaches full clock after ~3us of continuous execution, so the
emission order interleaves next-chunk qkv / previous-chunk projection
"filler" units between the score and PV matmuls of the attention inner
loop to keep the PE stream dense while ACT crunches the exps.
"""

import collections

import numpy as np
import ml_dtypes

import concourse.bass as bass  # noqa: F401
import concourse.tile as tile
from concourse import mybir, bacc
from concourse import bass_utils

f32 = mybir.dt.float32
f32r = mybir.dt.float32r
bf16 = mybir.dt.bfloat16
AL = mybir.AluOpType
AF = mybir.ActivationFunctionType


class _Bacc(bacc.Bacc):
    """Pin activations to the table set holding both ln and exp so the
    per-pass Ln<->Exp pair doesn't thrash ACT_TABLE_LOADs."""

    def insert_act_table_loads(self):
        import bass_rust as _bass_rust
        from concourse.hw_specs import get_activation_tables

        has_activation = any(
            isinstance(i, mybir.InstActivation)
            for bb in self.main_func.blocks
            for i in bb.instructions
        )
        if not has_activation:
            return
        tables = [
            (k, v if k == "natural_log_exp_and_others" else set())
            for k, v in get_activation_tables(self.m.arch).items()
        ]
        _bass_rust.insert_act_table_loads(self, tables)


HIDDEN = 1024
HEADS = 16
HD = 64
ROPE_BASE = 10000.0
N_CORES = 8
H8 = 8            # heads per core
NPAIR = 4         # head pairs per core
FQ = NPAIR * 128  # 512 q (or k, or v) feature columns per core
QCH = 512         # token chunk = attention q granule
DT = HIDDEN // 128  # 8 k-tiles for the qkv GEMM


def build_program(T):
    """Per-core program: one batch of T tokens, 8 heads (4 pairs)."""
    assert T % QCH == 0
    NCH = T // QCH
    NKT = T // 128
    nc = _Bacc("TRN2", target_bir_lowering=False, debug=False,
               num_devices=N_CORES)

    xT = nc.dram_tensor("xT", [HIDDEN, T], bf16, kind="ExternalInput").ap()
    w = nc.dram_tensor("w", [128, DT * 3 * FQ], bf16,
                       kind="ExternalInput").ap()
    bcol = nc.dram_tensor("bcol", [128, 12], f32, kind="ExternalInput").ap()
    psgn = nc.dram_tensor("psgn", [128, 128], bf16, kind="ExternalInput").ap()
    pw = nc.dram_tensor("pw", [128, NPAIR * HIDDEN], bf16,
                        kind="ExternalInput").ap()
    cos_t = nc.dram_tensor("cos_t", [128, T], bf16, kind="ExternalInput").ap()
    sin_t = nc.dram_tensor("sin_t", [128, T], bf16, kind="ExternalInput").ap()
    yT = nc.dram_tensor("yT", [HIDDEN, T], f32, kind="ExternalOutput").ap()

    FTW = 3 * FQ  # 1536 qkv feature columns per d-tile in w

    with tile.TileContext(nc) as tc:
        with tc.tile_pool(name="const", bufs=1) as constp, \
             tc.tile_pool(name="resident", bufs=1) as resp, \
             tc.tile_pool(name="xt", bufs=16) as xtp, \
             tc.tile_pool(name="rope", bufs=4) as ropep, \
             tc.tile_pool(name="qt", bufs=8) as qtp, \
             tc.tile_pool(name="pt", bufs=3) as ptp, \
             tc.tile_pool(name="ot", bufs=8) as otp, \
             tc.tile_pool(name="ysb", bufs=3) as yp, \
             tc.tile_pool(name="small", bufs=3) as smp, \
             tc.tile_pool(name="ps_s", bufs=2, space="PSUM") as ps_s_p, \
             tc.tile_pool(name="ps_o", bufs=2, space="PSUM") as ps_o_p, \
             tc.tile_pool(name="ps_m", bufs=2, space="PSUM") as ps_m_p:

            # ---- constants / residents ----
            w_sb = constp.tile([128, DT * FTW], bf16)
            nc.sync.dma_start(w_sb[:], w[:])
            bias_sb = constp.tile([128, 12], f32)
            nc.sync.dma_start(bias_sb[:], bcol[:])
            psgn_sb = constp.tile([128, 128], bf16)
            nc.sync.dma_start(psgn_sb[:], psgn[:])
            pw_sb = constp.tile([128, NPAIR * HIDDEN], bf16)
            nc.sync.dma_start(pw_sb[:], pw[:])
            cos_sb = constp.tile([128, T], bf16)
            nc.sync.dma_start(cos_sb[:], cos_t[:])
            sin_sb = constp.tile([128, T], bf16)
            nc.sync.dma_start(sin_sb[:], sin_t[:])
            ones_c = constp.tile([128, 2 * NKT], bf16)
            nc.gpsimd.memset(ones_c[:], 1.0)

            KT_res = [resp.tile([128, T], bf16, name=f"KT{p}")
                      for p in range(NPAIR)]
            # V layout per k-tile: [h0 v(64) | ones | pad(15) | h1 v(64) |
            # ones | pad(15)] -> the PV lhsT is a contiguous [128, 65] slice
            # whose 65th column emits the softmax row-sum.  The XBAR
            # dma-transpose only honors CONTIGUOUS, 16-element-ALIGNED
            # outputs on hardware, hence the per-head writes at offsets
            # 0 / 80 within the 160-element block.
            V_res = [resp.tile([128, NKT * 160], bf16, name=f"V{p}")
                     for p in range(NPAIR)]
            v4 = [V_res[p][:].rearrange("p (kt h c) -> p kt h c",
                                        kt=NKT, h=2) for p in range(NPAIR)]
            for p in range(NPAIR):
                nc.gpsimd.tensor_copy(
                    v4[p][:, :, :, 64],
                    ones_c[:].rearrange("p (kt h) -> p kt h", kt=NKT))

            ST = {}  # (qc, pair) -> QT tile;  ("ot", qc, pair) -> ot tile

            def emit_xt_loads(qc):
                t0 = qc * QCH
                xts = []
                for dt in range(DT):
                    t_ = xtp.tile([128, QCH], bf16, tag="xt", name=f"xt{dt}")
                    nc.sync.dma_start(t_[:],
                                      xT[dt * 128:(dt + 1) * 128, t0:t0 + QCH])
                    xts.append(t_)
                ST[("xt", qc)] = xts

            def qkv_units(qc):
                """Per-pair emission units for chunk qc's qkv+rope+V phase."""
                t0 = qc * QCH
                per_pair = [collections.deque() for _ in range(NPAIR)]
                for p in range(NPAIR):
                    units = per_pair[p]
                    for f in range(3):  # 0=q, 1=k, 2=v
                        ft = f * 4 + p
                        cell = {}

                        def unit_a(ft=ft, cell=cell, qc=qc):
                            ps_f = ps_m_p.tile([128, QCH], f32, tag="m",
                                               name="ps_f")
                            cell["ps"] = ps_f
                            xts = ST[("xt", qc)]
                            for dt in range(4):
                                nc.tensor.matmul(
                                    ps_f[:],
                                    w_sb[:, dt * FTW + ft * 128:
                                         dt * FTW + (ft + 1) * 128],
                                    xts[dt][:], start=(dt == 0), stop=False)

                        def unit_b(ft=ft, f=f, p=p, cell=cell, qc=qc, t0=t0):
                            ps_f = cell["ps"]
                            xts = ST[("xt", qc)]
                            for dt in range(4, DT):
                                nc.tensor.matmul(
                                    ps_f[:],
                                    w_sb[:, dt * FTW + ft * 128:
                                         dt * FTW + (ft + 1) * 128],
                                    xts[dt][:], start=False, stop=(dt == 7))
                            raw = ropep.tile([128, QCH], bf16, tag="raw",
                                             name="raw")
                            nc.vector.tensor_scalar_add(
                                raw[:], ps_f[:], bias_sb[:, ft:ft + 1])
                            if f < 2:
                                ps_rot = ps_m_p.tile([128, QCH], f32, tag="m",
                                                     name="ps_rot")
                                nc.tensor.matmul(ps_rot[:], psgn_sb[:], raw[:],
                                                 start=True, stop=True)
                                t1 = ropep.tile([128, QCH], bf16, tag="t1",
                                                name="t1")
                                nc.vector.tensor_tensor(
                                    t1[:], raw[:], cos_sb[:, t0:t0 + QCH],
                                    AL.mult)
                                t2 = ropep.tile([128, QCH], bf16, tag="t2",
                                                name="t2")
                                nc.vector.tensor_tensor(
                                    t2[:], ps_rot[:], sin_sb[:, t0:t0 + QCH],
                                    AL.mult)
                                if f == 0:
                                    qt_ = qtp.tile([128, QCH], bf16, tag="qt",
                                                   name=f"QT{p}")
                                    ST[(qc, p)] = qt_
                                    dst = qt_[:]
                                else:
                                    dst = KT_res[p][:, t0:t0 + QCH]
                                nc.vector.tensor_tensor(dst, t1[:], t2[:],
                                                        AL.add)
                            else:
                                kt0 = t0 // 128
                                for tk in range(4):
                                    for h2 in range(2):
                                        nc.sync.dma_start_transpose(
                                            v4[p][:, kt0 + tk, h2, 0:64],
                                            raw[64 * h2:64 * h2 + 64,
                                                tk * 128:(tk + 1) * 128])

                        units.append(unit_a)
                        units.append(unit_b)
                return per_pair

            def proj_units(qc):
                """Emission units for chunk qc's projection (needs norm)."""
                t0 = qc * QCH
                units = collections.deque()
                ots = [ST[("ot", qc, p)] for p in range(NPAIR)]
                for oi in range(8):
                    def unit(oi=oi, ots=ots, t0=t0):
                        ps_y = ps_m_p.tile([128, QCH], f32, tag="m",
                                           name="ps_y")
                        for kt in range(NPAIR):
                            nc.tensor.matmul(
                                ps_y[:],
                                pw_sb[:, kt * HIDDEN + oi * 128:
                                      kt * HIDDEN + (oi + 1) * 128],
                                ots[kt][:], start=(kt == 0),
                                stop=(kt == NPAIR - 1))
                        ysb = yp.tile([128, QCH], f32, tag="y", name="ysb")
                        if oi % 2 == 0:
                            nc.vector.tensor_copy(ysb[:], ps_y[:])
                        else:
                            nc.scalar.copy(ysb[:], ps_y[:])
                        nc.sync.dma_start(
                            yT[oi * 128:(oi + 1) * 128, t0:t0 + QCH], ysb[:])
                    units.append(unit)
                return units

            def attention_pass(qc, p, fillers):
                """One head-pair's attention for chunk qc.  `fillers` is a
                list of deques of emission units, pulled between the score
                and PV matmuls to keep the PE stream dense."""
                Q0 = qc * QCH
                nkb = (Q0 + QCH) // 128

                def pull(n=1):
                    for _ in range(n):
                        for fl in fillers:
                            if fl:
                                fl.popleft()()
                                break

                O = [ps_o_p.tile([65, QCH], f32, tag="o",
                                 name=f"O{h2}") for h2 in range(2)]
                for kb in range(nkb):
                    qs = max(0, 128 * kb - Q0)
                    ps_sc = ps_s_p.tile([128, 2 * QCH], f32, tag="s",
                                        name="ps_sc")
                    for h2 in range(2):
                        hp = slice(64 * h2, 64 * h2 + 64)
                        nc.tensor.matmul(
                            ps_sc[:, QCH * h2 + qs:QCH * h2 + QCH],
                            KT_res[p][hp, kb * 128:(kb + 1) * 128],
                            ST[(qc, p)][hp, qs:QCH],
                            start=True, stop=True)
                    pt = ptp.tile([128, 2 * QCH], bf16, tag="pt", name="pt")
                    sc4 = ps_sc[:].rearrange("p (h q) -> p h q", h=2)
                    pt4 = pt[:].rearrange("p (h q) -> p h q", h=2)
                    nc.scalar.activation(pt4[:, :, qs:QCH],
                                         sc4[:, :, qs:QCH],
                                         AF.Exp, bias=0.0, scale=0.125)
                    if 128 * kb >= Q0:
                        ds = 128 * kb - Q0
                        for h2 in range(2):
                            nc.gpsimd.affine_select(
                                out=pt[:, QCH * h2 + ds:QCH * h2 + ds + 128],
                                in_=pt[:, QCH * h2 + ds:QCH * h2 + ds + 128],
                                pattern=[[1, 128]], compare_op=AL.is_ge,
                                fill=0.0, base=0, channel_multiplier=-1)
                    pull(3 if kb == 0 else 1)
                    for h2 in range(2):
                        nc.tensor.matmul(
                            O[h2][:, qs:QCH],
                            v4[p][:, kb, h2, 0:65],
                            pt[:, QCH * h2 + qs:QCH * h2 + QCH],
                            start=(kb == 0), stop=(kb == nkb - 1))
                # normalize (1/rowsum via exp(-ln)); chains of the two heads
                # interleaved so the O banks free as early as possible (the
                # next pass's first PV has a WAR on them)
                ot_ = otp.tile([128, QCH], bf16, tag="ot", name=f"ot{p}")
                ST[("ot", qc, p)] = ot_
                lnv = [smp.tile([1, QCH], f32, tag=f"ln{h2}", name="lnv")
                       for h2 in range(2)]
                rs = [smp.tile([1, QCH], f32, tag=f"rs{h2}", name="rs")
                      for h2 in range(2)]
                rsb = [smp.tile([64, QCH], f32, tag=f"rsb{h2}", name="rsb")
                       for h2 in range(2)]
                for h2 in range(2):
                    nc.scalar.activation(lnv[h2][:], O[h2][64:65, :], AF.Ln)
                for h2 in range(2):
                    nc.scalar.activation(rs[h2][:], lnv[h2][:], AF.Exp,
                                         bias=0.0, scale=-1.0)
                for h2 in range(2):
                    nc.gpsimd.partition_broadcast(rsb[h2][:], rs[h2][:])
                for h2 in range(2):
                    nc.vector.tensor_tensor(
                        ot_[64 * h2:64 * h2 + 64, :], O[h2][0:64, :],
                        rsb[h2][:], AL.mult)

            # ---------------- main schedule ----------------
            # attention pass (qc, p) pulls, in priority order: the same
            # chunk's next-pair qkv units, then (in the last pass) the next
            # chunk's pair-0 units, then the previous chunk's projection.
            emit_xt_loads(0)
            U = qkv_units(0)
            for u in U[0]:
                u()
            U[0].clear()
            fp = collections.deque()
            for qc in range(NCH):
                if qc + 1 < NCH:
                    emit_xt_loads(qc + 1)
                    Un = qkv_units(qc + 1)
                else:
                    Un = [collections.deque() for _ in range(NPAIR)]
                for p in range(NPAIR):
                    for u in U[p]:  # force-drain this pair's leftovers
                        u()
                    U[p].clear()
                    prim = U[p + 1] if p + 1 < NPAIR else Un[0]
                    attention_pass(qc, p, [prim, fp])
                U = Un
                while fp:
                    fp.popleft()()
                fp = proj_units(qc)
            while fp:
                fp.popleft()()
    nc.compile()
    return nc


# ---------------------------------------------------------------- host side

def _rope_tables(T):
    inv_freq = 1.0 / (ROPE_BASE ** (np.arange(0, HD, 2, dtype=np.float64)
                                    / HD))
    pos = np.arange(T, dtype=np.float64)
    ang = np.outer(pos, inv_freq)          # [T, 32]
    cos = np.cos(ang)
    sin = np.sin(ang)
    jm = (np.arange(128) % 64) % 32
    cos_t = np.ascontiguousarray(cos[:, jm].T).astype(ml_dtypes.bfloat16)
    sin_t = np.ascontiguousarray(sin[:, jm].T).astype(ml_dtypes.bfloat16)
    return cos_t, sin_t


def _psgn():
    p = np.zeros((HD, HD), np.float32)
    for i in range(32):
        p[i + 32, i] = -1.0   # out dim i (<32) = -in dim i+32
        p[i, i + 32] = 1.0    # out dim i+32   = +in dim i
    pf = np.zeros((128, 128), np.float32)
    pf[0:64, 0:64] = p        # head-even block
    pf[64:128, 64:128] = p    # head-odd block
    return np.ascontiguousarray(pf).astype(ml_dtypes.bfloat16)


def make_core_inputs(x, qkv_w, qkv_b, proj_w, B, T):
    x = np.asarray(x, dtype=np.float32)
    qkv_w = np.asarray(qkv_w, dtype=np.float32)
    qkv_b = np.asarray(qkv_b, dtype=np.float32)
    proj_w = np.asarray(proj_w, dtype=np.float32)
    cos_t, sin_t = _rope_tables(T)
    psgn = _psgn()
    xTs = [np.ascontiguousarray(x[b].T).astype(ml_dtypes.bfloat16)
           for b in range(B)]
    in_maps = []
    for c in range(N_CORES):
        b, g = divmod(c, 2)
        col = FQ * g
        wc = np.concatenate(
            [qkv_w[:, col:col + FQ],
             qkv_w[:, HIDDEN + col:HIDDEN + col + FQ],
             qkv_w[:, 2 * HIDDEN + col:2 * HIDDEN + col + FQ]],
            axis=1)                                     # [1024, 1536]
        wc = np.ascontiguousarray(
            wc.reshape(DT, 128, 3 * FQ).transpose(1, 0, 2).reshape(
                128, DT * 3 * FQ)).astype(ml_dtypes.bfloat16)
        bc = np.zeros((128, 12), np.float32)
        for f in range(3):
            for p in range(NPAIR):
                bc[:, f * 4 + p] = qkv_b[f * HIDDEN + col + 128 * p:
                                         f * HIDDEN + col + 128 * (p + 1)]
        pwc = np.ascontiguousarray(
            proj_w[col:col + FQ, :].reshape(NPAIR, 128, HIDDEN)
            .transpose(1, 0, 2).reshape(128, NPAIR * HIDDEN)
        ).astype(ml_dtypes.bfloat16)
        in_maps.append({
            "xT": xTs[b], "w": wc, "bcol": np.ascontiguousarray(bc),
            "psgn": psgn, "pw": pwc, "cos_t": cos_t, "sin_t": sin_t,
        })
    return in_maps


_PROGRAM_CACHE = {}


def _get_program(T):
    if T not in _PROGRAM_CACHE:
        _PROGRAM_CACHE[T] = build_program(T)
    return _PROGRAM_CACHE[T]


def run(x, qkv_w, qkv_b, proj_w, proj_b, NB, T, trace=False):
    nc = _get_program(T)
    in_maps = make_core_inputs(x, qkv_w, qkv_b, proj_w, NB, T)
    res = bass_utils.run_bass_kernel_spmd(
        nc, in_maps, core_ids=list(range(N_CORES)), trace=trace)
    pb = np.asarray(proj_b, dtype=np.float32)[None, None, :]
    out = np.empty((NB, T, HIDDEN), np.float32)
    for b in range(NB):
        acc = res.results[2 * b]["yT"].astype(np.float32) \
            + res.results[2 * b + 1]["yT"]
        out[b] = acc.T
    out += pb
    return out, res


def kernel(x, qkv_w, qkv_b, proj_w, proj_b):
    x = np.asarray(x)
    B, L, D = x.shape
    assert D == HIDDEN and B % 2 == 0 and N_CORES == 2 * B
    out, _ = run(x, np.asarray(qkv_w), np.asarray(qkv_b),
                 np.asarray(proj_w), np.asarray(proj_b), NB=B, T=L)
    return out.astype(np.float32)



# revision 3
# speedup vs baseline: 1.1370x; 1.1370x over previous
"""Trainium2 Bass kernel for nn_CausalSelfAttention (B=4, L=2048, D=1024, H=16).

Sharding: batch x head-group.  Core c handles batch c//2 and heads
8*(c%2) .. 8*(c%2)+8 (8 heads = 4 pairs), i.e. 2 cores per batch, each
producing a partial projection yT_c = proj_w[rows_c].T @ O_c^T of shape
[D, T].  The host sums the 2 partials per batch, transposes, adds an
effective proj bias (proj_b + v_bias @ proj_w -- exact because softmax
rows sum to 1, so the v-bias passes through attention unchanged).

All matmul operands are bf16 (PSUM accumulation stays fp32).  x is
transposed to [D, T] on the host, so the device does ZERO transposes:
V is computed directly token-major on the PE by swapping the matmul
operands (VT[tok, feat] = xt.T @ w_v instead of w_v.T @ xt), which
removes the XBAR dma-transpose chain that serialized one DMA queue and
stalled the PE at every chunk boundary in the previous version.

Device pipeline per core (per 512-token chunk):
  xt tiles [d, tok] <- plain DMA from host-transposed xT
  q/k: qT = w.T @ xt (8 K-tiles, fp32 PSUM), bias folded into the
      PSUM->SBUF copy via DVE tensor_scalar_add, then RoPE (psgn matmul
      for rotate_half + cos/sin muls on DVE)
  V: VT[tok, 512 feats] = xt.T @ wv (8 K-tiles), PSUM->SBUF copy
      scatters into a [tok, kb, pair, h2, 80] resident layout whose
      65th column per head is pre-set to ones -> the PV matmul emits
      softmax row-sums for free
  attention per head-pair: S^T = KT.T @ QT (two heads row-tiled into
      one PE slot) -> exp on ACT (scale=1/8, bf16 out) -> causal via
      affine_select on diagonal blocks -> O[65, 2*512] += [V|1].T @ P^T
      (both heads accumulate into ONE 2-bank PSUM tile)
  normalize: one wide Ln + one wide Exp(-x) on [1, 1024] (both heads),
      one gpsimd partition_broadcast, DVE muls -> ot bf16
  proj: yT += pw.T @ ot (4 K-tiles per out tile), copies on DVE,
      output DMA on the ACT hardware queue (inputs use the SYNC queue)

Scheduling: engines execute their queues in order, and the PE p-state
only reaches full clock after ~3us of continuous execution, so the
emission order interleaves next-chunk qkv / previous-chunk projection
"filler" units between the score and PV matmuls of the attention inner
loop to keep the PE stream dense while ACT crunches the exps.  The VT
units of a chunk are emitted just-in-time ahead of the diagonal PV
blocks that consume them.
"""

import collections

import numpy as np
import ml_dtypes

import concourse.bass as bass  # noqa: F401
import concourse.tile as tile
from concourse import mybir, bacc
from concourse import bass_utils

f32 = mybir.dt.float32
bf16 = mybir.dt.bfloat16
AL = mybir.AluOpType
AF = mybir.ActivationFunctionType


class _Bacc(bacc.Bacc):
    """Pin activations to the table set holding both ln and exp so the
    per-pass Ln<->Exp pair doesn't thrash ACT_TABLE_LOADs."""

    def insert_act_table_loads(self):
        import bass_rust as _bass_rust
        from concourse.hw_specs import get_activation_tables

        has_activation = any(
            isinstance(i, mybir.InstActivation)
            for bb in self.main_func.blocks
            for i in bb.instructions
        )
        if not has_activation:
            return
        tables = [
            (k, v if k == "natural_log_exp_and_others" else set())
            for k, v in get_activation_tables(self.m.arch).items()
        ]
        _bass_rust.insert_act_table_loads(self, tables)


HIDDEN = 1024
HEADS = 16
HD = 64
ROPE_BASE = 10000.0
N_CORES = 8
H8 = 8            # heads per core
NPAIR = 4         # head pairs per core
FQ = NPAIR * 128  # 512 q (or k, or v) feature columns per core
QCH = 512         # token chunk = attention q granule
DT = HIDDEN // 128  # 8 k-tiles for the qkv GEMM


def build_program(T):
    """Per-core program: one batch of T tokens, 8 heads (4 pairs)."""
    assert T % QCH == 0
    NCH = T // QCH
    NKT = T // 128
    nc = _Bacc("TRN2", target_bir_lowering=False, debug=False,
               num_devices=N_CORES)

    xT = nc.dram_tensor("xT", [HIDDEN, T], bf16, kind="ExternalInput").ap()
    # q/k weights, ft-major: [128, ft(8) x dt(8) x 128]; ft = f*4+pair
    wqk = nc.dram_tensor("wqk", [128, 8 * DT * 128], bf16,
                         kind="ExternalInput").ap()
    # v weights, dt-major: [128, dt(8) x 512]
    wv = nc.dram_tensor("wv", [128, DT * FQ], bf16,
                        kind="ExternalInput").ap()
    bcol = nc.dram_tensor("bcol", [128, 8], f32, kind="ExternalInput").ap()
    psgn = nc.dram_tensor("psgn", [128, 128], bf16, kind="ExternalInput").ap()
    pw = nc.dram_tensor("pw", [128, NPAIR * HIDDEN], bf16,
                        kind="ExternalInput").ap()
    cos_t = nc.dram_tensor("cos_t", [128, T], bf16, kind="ExternalInput").ap()
    sin_t = nc.dram_tensor("sin_t", [128, T], bf16, kind="ExternalInput").ap()
    yT = nc.dram_tensor("yT", [HIDDEN, T], f32, kind="ExternalOutput").ap()

    with tile.TileContext(nc) as tc:
        with tc.tile_pool(name="const", bufs=1) as constp, \
             tc.tile_pool(name="resident", bufs=1) as resp, \
             tc.tile_pool(name="xt", bufs=16) as xtp, \
             tc.tile_pool(name="rope", bufs=4) as ropep, \
             tc.tile_pool(name="qt", bufs=8) as qtp, \
             tc.tile_pool(name="pt", bufs=5) as ptp, \
             tc.tile_pool(name="ot", bufs=8) as otp, \
             tc.tile_pool(name="ysb", bufs=3) as yp, \
             tc.tile_pool(name="small", bufs=2) as smp, \
             tc.tile_pool(name="ps_s", bufs=2, space="PSUM") as ps_s_p, \
             tc.tile_pool(name="ps_o", bufs=1, space="PSUM") as ps_o_p, \
             tc.tile_pool(name="ps_m", bufs=2, space="PSUM") as ps_m_p:

            # ---- constants / residents ----
            # Emission order of the input DMAs is their queue order; the
            # first qkv units need wqk[ft=0] (q, pair 0) and wqk[ft=4]
            # (k, pair 0) plus xt -- those go first on the SYNC queue.
            # Small constants stream in parallel on the ACT queue.
            wqk_sb = [None] * 8
            for ft in (0, 4):
                wqk_sb[ft] = constp.tile([128, DT * 128], bf16,
                                         name=f"wqk{ft}")
                nc.sync.dma_start(wqk_sb[ft][:],
                                  wqk[:, ft * DT * 128:(ft + 1) * DT * 128])
            bias_sb = constp.tile([128, 8], f32)
            nc.scalar.dma_start(bias_sb[:], bcol[:])
            psgn_sb = constp.tile([128, 128], bf16)
            nc.scalar.dma_start(psgn_sb[:], psgn[:])
            cos_sb = constp.tile([128, T], bf16)
            nc.scalar.dma_start(cos_sb[:], cos_t[:])
            sin_sb = constp.tile([128, T], bf16)
            nc.scalar.dma_start(sin_sb[:], sin_t[:])

            ST = {}  # (qc, pair) -> QT tile;  ("ot", qc, pair) -> ot tile

            def emit_xt_loads(qc):
                t0 = qc * QCH
                xts = []
                for dt in range(DT):
                    t_ = xtp.tile([128, QCH], bf16, tag="xt", name=f"xt{dt}")
                    nc.sync.dma_start(t_[:],
                                      xT[dt * 128:(dt + 1) * 128, t0:t0 + QCH])
                    xts.append(t_)
                ST[("xt", qc)] = xts

            emit_xt_loads(0)

            wv_sb = constp.tile([128, DT * FQ], bf16, name="wv")
            nc.scalar.dma_start(wv_sb[:], wv[:])
            pw_sb = constp.tile([128, NPAIR * HIDDEN], bf16, name="pw")
            nc.scalar.dma_start(pw_sb[:], pw[:])
            for ft in (1, 5, 2, 6, 3, 7):
                wqk_sb[ft] = constp.tile([128, DT * 128], bf16,
                                         name=f"wqk{ft}")
                nc.sync.dma_start(wqk_sb[ft][:],
                                  wqk[:, ft * DT * 128:(ft + 1) * DT * 128])

            ones_c = constp.tile([128, 128], bf16)
            nc.gpsimd.memset(ones_c[:], 1.0)

            KT_res = [resp.tile([128, T], bf16, name=f"KT{p}")
                      for p in range(NPAIR)]
            # V resident, token-major: [tok, kb, pair, h2, 80] where the
            # 80-block is [v(64) | ones | pad(15)] -> the PV lhsT is a
            # contiguous [128, 65] slice whose 65th column emits the
            # softmax row-sum.
            VT_res = resp.tile([128, NKT * NPAIR * 2 * 80], bf16, name="VT")
            VT5 = VT_res[:].rearrange("t (kb p h c) -> t kb p h c",
                                      kb=NKT, p=NPAIR, h=2)
            nc.gpsimd.tensor_copy(
                VT5[:, :, :, :, 64],
                ones_c[:].rearrange("t (a b h) -> t a b h", a=NKT, b=NPAIR))

            def qk_units(qc):
                """Per-pair q/k emission units for chunk qc."""
                t0 = qc * QCH
                per_pair = [collections.deque() for _ in range(NPAIR)]
                for p in range(NPAIR):
                    units = per_pair[p]
                    for f in range(2):  # 0=q, 1=k
                        ft = f * 4 + p
                        cell = {}

                        def unit_a(ft=ft, cell=cell, qc=qc):
                            ps_f = ps_m_p.tile([128, QCH], f32, tag="m",
                                               name="ps_f")
                            cell["ps"] = ps_f
                            xts = ST[("xt", qc)]
                            for dt in range(4):
                                nc.tensor.matmul(
                                    ps_f[:],
                                    wqk_sb[ft][:, dt * 128:(dt + 1) * 128],
                                    xts[dt][:], start=(dt == 0), stop=False)

                        def unit_b(ft=ft, f=f, p=p, cell=cell, qc=qc, t0=t0):
                            ps_f = cell["ps"]
                            xts = ST[("xt", qc)]
                            for dt in range(4, DT):
                                nc.tensor.matmul(
                                    ps_f[:],
                                    wqk_sb[ft][:, dt * 128:(dt + 1) * 128],
                                    xts[dt][:], start=False, stop=(dt == 7))
                            raw = ropep.tile([128, QCH], bf16, tag="raw",
                                             name="raw")
                            nc.vector.tensor_scalar_add(
                                raw[:], ps_f[:], bias_sb[:, ft:ft + 1])
                            ps_rot = ps_m_p.tile([128, QCH], f32, tag="m",
                                                 name="ps_rot")
                            nc.tensor.matmul(ps_rot[:], psgn_sb[:], raw[:],
                                             start=True, stop=True)
                            t1 = ropep.tile([128, QCH], bf16, tag="t1",
                                            name="t1")
                            nc.vector.tensor_tensor(
                                t1[:], raw[:], cos_sb[:, t0:t0 + QCH],
                                AL.mult)
                            t2 = ropep.tile([128, QCH], bf16, tag="t2",
                                            name="t2")
                            nc.vector.tensor_tensor(
                                t2[:], ps_rot[:], sin_sb[:, t0:t0 + QCH],
                                AL.mult)
                            if f == 0:
                                qt_ = qtp.tile([128, QCH], bf16, tag="qt",
                                               name=f"QT{p}")
                                ST[(qc, p)] = qt_
                                dst = qt_[:]
                            else:
                                dst = KT_res[p][:, t0:t0 + QCH]
                            nc.vector.tensor_tensor(dst, t1[:], t2[:],
                                                    AL.add)

                        units.append(unit_a)
                        units.append(unit_b)
                return per_pair

            def vt_units(qc):
                """VT emission units for chunk qc: per 128-token block tb,
                VT[tok, feat] = xt.T @ wv accumulated over 8 d-tiles."""
                kt0 = qc * QCH // 128
                units = collections.deque()
                for tb in range(4):
                    cell = {}

                    def unit_a(tb=tb, cell=cell, qc=qc):
                        ps_v = ps_m_p.tile([128, FQ], f32, tag="m",
                                           name="ps_v")
                        cell["ps"] = ps_v
                        xts = ST[("xt", qc)]
                        for dt in range(4):
                            nc.tensor.matmul(
                                ps_v[:],
                                xts[dt][:, tb * 128:(tb + 1) * 128],
                                wv_sb[:, dt * FQ:(dt + 1) * FQ],
                                start=(dt == 0), stop=False)

                    def unit_b(tb=tb, cell=cell, qc=qc, kt0=kt0):
                        ps_v = cell["ps"]
                        xts = ST[("xt", qc)]
                        for dt in range(4, DT):
                            nc.tensor.matmul(
                                ps_v[:],
                                xts[dt][:, tb * 128:(tb + 1) * 128],
                                wv_sb[:, dt * FQ:(dt + 1) * FQ],
                                start=False, stop=(dt == 7))
                        nc.vector.tensor_copy(
                            VT5[:, kt0 + tb, :, :, 0:64],
                            ps_v[:].rearrange("t (p h c) -> t p h c",
                                              p=NPAIR, h=2))

                    units.append(unit_a)
                    units.append(unit_b)
                return units

            def proj_units(qc):
                """Emission units for chunk qc's projection (needs norm)."""
                t0 = qc * QCH
                units = collections.deque()
                ots = [ST[("ot", qc, p)] for p in range(NPAIR)]
                for oi in range(8):
                    def unit(oi=oi, ots=ots, t0=t0):
                        ps_y = ps_m_p.tile([128, QCH], f32, tag="m",
                                           name="ps_y")
                        for kt in range(NPAIR):
                            nc.tensor.matmul(
                                ps_y[:],
                                pw_sb[:, kt * HIDDEN + oi * 128:
                                      kt * HIDDEN + (oi + 1) * 128],
                                ots[kt][:], start=(kt == 0),
                                stop=(kt == NPAIR - 1))
                        ysb = yp.tile([128, QCH], f32, tag="y", name="ysb")
                        nc.vector.tensor_copy(ysb[:], ps_y[:])
                        nc.scalar.dma_start(
                            yT[oi * 128:(oi + 1) * 128, t0:t0 + QCH], ysb[:])
                    units.append(unit)
                return units

            def attention_pass(qc, p, fillers):
                """One head-pair's attention for chunk qc.  `fillers` is a
                list of deques of emission units, pulled between the score
                and PV matmuls to keep the PE stream dense."""
                Q0 = qc * QCH
                nkb = (Q0 + QCH) // 128

                def pull(n=1):
                    for _ in range(n):
                        for fl in fillers:
                            if fl:
                                fl.popleft()()
                                break

                # both heads' O (and their ones-column row sums at
                # partition 64) accumulate into ONE 2-bank PSUM tile:
                # O[:, h2*QCH + q]
                O = ps_o_p.tile([65, 2 * QCH], f32, tag="o", name="O")
                for kb in range(nkb):
                    qs = max(0, 128 * kb - Q0)
                    ps_sc = ps_s_p.tile([128, 2 * QCH], f32, tag="s",
                                        name="ps_sc")
                    for h2 in range(2):
                        hp = slice(64 * h2, 64 * h2 + 64)
                        nc.tensor.matmul(
                            ps_sc[:, QCH * h2 + qs:QCH * h2 + QCH],
                            KT_res[p][hp, kb * 128:(kb + 1) * 128],
                            ST[(qc, p)][hp, qs:QCH],
                            start=True, stop=True)
                    pt = ptp.tile([128, 2 * QCH], bf16, tag="pt", name="pt")
                    sc4 = ps_sc[:].rearrange("p (h q) -> p h q", h=2)
                    pt4 = pt[:].rearrange("p (h q) -> p h q", h=2)
                    nc.scalar.activation(pt4[:, :, qs:QCH],
                                         sc4[:, :, qs:QCH],
                                         AF.Exp, bias=0.0, scale=0.125)
                    if 128 * kb >= Q0:
                        ds = 128 * kb - Q0
                        for h2 in range(2):
                            nc.gpsimd.affine_select(
                                out=pt[:, QCH * h2 + ds:QCH * h2 + ds + 128],
                                in_=pt[:, QCH * h2 + ds:QCH * h2 + ds + 128],
                                pattern=[[1, 128]], compare_op=AL.is_ge,
                                fill=0.0, base=0, channel_multiplier=-1)
                    pull(3 if kb == 0 else 1)
                    for h2 in range(2):
                        nc.tensor.matmul(
                            O[:, QCH * h2 + qs:QCH * h2 + QCH],
                            VT5[:, kb, p, h2, 0:65],
                            pt[:, QCH * h2 + qs:QCH * h2 + QCH],
                            start=(kb == 0), stop=(kb == nkb - 1))
                # normalize: 1/rowsum via exp(-ln(rowsum)), one wide pass
                # over both heads' row-sum rows (partition 64 of O)
                ot_ = otp.tile([128, QCH], bf16, tag="ot", name=f"ot{p}")
                ST[("ot", qc, p)] = ot_
                lnv = smp.tile([1, 2 * QCH], f32, tag="ln", name="lnv")
                nc.scalar.activation(lnv[:], O[64:65, :], AF.Ln)
                rs = smp.tile([1, 2 * QCH], f32, tag="rs", name="rs")
                nc.scalar.activation(rs[:], lnv[:], AF.Exp,
                                     bias=0.0, scale=-1.0)
                rsb = smp.tile([64, 2 * QCH], f32, tag="rsb", name="rsb")
                nc.gpsimd.partition_broadcast(rsb[:], rs[:])
                for h2 in range(2):
                    nc.vector.tensor_tensor(
                        ot_[64 * h2:64 * h2 + 64, :],
                        O[0:64, QCH * h2:QCH * h2 + QCH],
                        rsb[:, QCH * h2:QCH * h2 + QCH], AL.mult)

            # ---------------- main schedule ----------------
            # attention pass (qc, p) pulls, in priority order: this
            # chunk's remaining VT units (just-in-time for the diagonal
            # PV blocks), the same chunk's next-pair q/k units, then (in
            # the last pass) the next chunk's pair-0 units, then the
            # previous chunk's projection.
            U = qk_units(0)
            V = vt_units(0)
            # bootstrap: pair-0 q/k plus the first VT block before the
            # first attention pass
            for u in U[0]:
                u()
            U[0].clear()
            for _ in range(2):
                V.popleft()()
            fp = collections.deque()
            for qc in range(NCH):
                if qc + 1 < NCH:
                    emit_xt_loads(qc + 1)
                    Un = qk_units(qc + 1)
                    Vn = vt_units(qc + 1)
                else:
                    Un = [collections.deque() for _ in range(NPAIR)]
                    Vn = collections.deque()
                for p in range(NPAIR):
                    for u in U[p]:  # force-drain this pair's leftovers
                        u()
                    U[p].clear()
                    if p == 1:
                        while V:  # VT must be emitted before pass 1's
                            V.popleft()()  # diagonal PVs at the latest
                    if p + 1 < NPAIR:
                        fillers = [V, U[p + 1], fp]
                    else:
                        fillers = [V, Un[0], Vn, fp]
                    attention_pass(qc, p, fillers)
                U = Un
                # next chunk's VT: first block force-emitted so pass
                # (qc+1, 0)'s kb0 PV never waits on emission order
                if Vn:
                    for _ in range(2):
                        Vn.popleft()()
                V = Vn
                while fp:
                    fp.popleft()()
                fp = proj_units(qc)
            while fp:
                fp.popleft()()
    nc.compile()
    return nc


# ---------------------------------------------------------------- host side

def _rope_tables(T):
    inv_freq = 1.0 / (ROPE_BASE ** (np.arange(0, HD, 2, dtype=np.float64)
                                    / HD))
    pos = np.arange(T, dtype=np.float64)
    ang = np.outer(pos, inv_freq)          # [T, 32]
    cos = np.cos(ang)
    sin = np.sin(ang)
    jm = (np.arange(128) % 64) % 32
    cos_t = np.ascontiguousarray(cos[:, jm].T).astype(ml_dtypes.bfloat16)
    sin_t = np.ascontiguousarray(sin[:, jm].T).astype(ml_dtypes.bfloat16)
    return cos_t, sin_t


def _psgn():
    p = np.zeros((HD, HD), np.float32)
    for i in range(32):
        p[i + 32, i] = -1.0   # out dim i (<32) = -in dim i+32
        p[i, i + 32] = 1.0    # out dim i+32   = +in dim i
    pf = np.zeros((128, 128), np.float32)
    pf[0:64, 0:64] = p        # head-even block
    pf[64:128, 64:128] = p    # head-odd block
    return np.ascontiguousarray(pf).astype(ml_dtypes.bfloat16)


def make_core_inputs(x, qkv_w, qkv_b, proj_w, B, T):
    x = np.asarray(x, dtype=np.float32)
    qkv_w = np.asarray(qkv_w, dtype=np.float32)
    qkv_b = np.asarray(qkv_b, dtype=np.float32)
    proj_w = np.asarray(proj_w, dtype=np.float32)
    cos_t, sin_t = _rope_tables(T)
    psgn = _psgn()
    xTs = [np.ascontiguousarray(x[b].T).astype(ml_dtypes.bfloat16)
           for b in range(B)]
    in_maps = []
    for c in range(N_CORES):
        b, g = divmod(c, 2)
        col = FQ * g
        # q/k weights, ft-major: ft = f*4 + pair
        blocks = []
        for f in range(2):
            for p in range(NPAIR):
                sl = qkv_w[:, f * HIDDEN + col + 128 * p:
                           f * HIDDEN + col + 128 * (p + 1)]      # [1024,128]
                blocks.append(sl.reshape(DT, 128, 128)
                              .transpose(1, 0, 2).reshape(128, DT * 128))
        wqk_c = np.ascontiguousarray(np.concatenate(blocks, axis=1)) \
            .astype(ml_dtypes.bfloat16)                           # [128,8192]
        # v weights, dt-major
        slv = qkv_w[:, 2 * HIDDEN + col:2 * HIDDEN + col + FQ]    # [1024,512]
        wv_c = np.ascontiguousarray(
            slv.reshape(DT, 128, FQ).transpose(1, 0, 2)
            .reshape(128, DT * FQ)).astype(ml_dtypes.bfloat16)
        bc = np.zeros((128, 8), np.float32)
        for f in range(2):
            for p in range(NPAIR):
                bc[:, f * 4 + p] = qkv_b[f * HIDDEN + col + 128 * p:
                                         f * HIDDEN + col + 128 * (p + 1)]
        pwc = np.ascontiguousarray(
            proj_w[col:col + FQ, :].reshape(NPAIR, 128, HIDDEN)
            .transpose(1, 0, 2).reshape(128, NPAIR * HIDDEN)
        ).astype(ml_dtypes.bfloat16)
        in_maps.append({
            "xT": xTs[b], "wqk": wqk_c, "wv": wv_c,
            "bcol": np.ascontiguousarray(bc),
            "psgn": psgn, "pw": pwc, "cos_t": cos_t, "sin_t": sin_t,
        })
    return in_maps


_PROGRAM_CACHE = {}


def _get_program(T):
    if T not in _PROGRAM_CACHE:
        _PROGRAM_CACHE[T] = build_program(T)
    return _PROGRAM_CACHE[T]


def run(x, qkv_w, qkv_b, proj_w, proj_b, NB, T, trace=False):
    nc = _get_program(T)
    in_maps = make_core_inputs(x, qkv_w, qkv_b, proj_w, NB, T)
    res = bass_utils.run_bass_kernel_spmd(
        nc, in_maps, core_ids=list(range(N_CORES)), trace=trace)
    # v-bias passes through attention unchanged (softmax rows sum to 1),
    # so fold it into an effective proj bias on the host -- exact.
    qkv_b = np.asarray(qkv_b, dtype=np.float32)
    pb_eff = (np.asarray(proj_b, dtype=np.float32)
              + qkv_b[2 * HIDDEN:] @ np.asarray(proj_w, dtype=np.float32))
    out = np.empty((NB, T, HIDDEN), np.float32)
    for b in range(NB):
        acc = res.results[2 * b]["yT"].astype(np.float32) \
            + res.results[2 * b + 1]["yT"]
        out[b] = acc.T
    out += pb_eff[None, None, :]
    return out, res


def kernel(x, qkv_w, qkv_b, proj_w, proj_b):
    x = np.asarray(x)
    B, L, D = x.shape
    assert D == HIDDEN and B % 2 == 0 and N_CORES == 2 * B
    out, _ = run(x, np.asarray(qkv_w), np.asarray(qkv_b),
                 np.asarray(proj_w), np.asarray(proj_b), NB=B, T=L)
    return out.astype(np.float32)


# revision 7
# speedup vs baseline: 1.1473x; 1.0090x over previous
"""Trainium2 Bass kernel for nn_CausalSelfAttention (B=4, L=2048, D=1024, H=16).

Sharding: batch x head-group.  Core c handles batch c//2 and heads
8*(c%2) .. 8*(c%2)+8 (8 heads = 4 pairs), i.e. 2 cores per batch, each
producing a partial projection yT_c = proj_w[rows_c].T @ O_c^T of shape
[D, T].  The host sums the 2 partials per batch, transposes, adds an
effective proj bias (proj_b + v_bias @ proj_w -- exact because softmax
rows sum to 1, so the v-bias passes through attention unchanged).

All matmul operands are bf16 (PSUM accumulation stays fp32).  x is
transposed to [D, T] on the host, so the device does ZERO transposes:
V is computed directly token-major on the PE by swapping the matmul
operands (VT[tok, feat] = xt.T @ w_v instead of w_v.T @ xt), which
removes the XBAR dma-transpose chain that serialized one DMA queue and
stalled the PE at every chunk boundary in the previous version.

Device pipeline per core (per 512-token chunk):
  xt tiles [d, tok] <- plain DMA from host-transposed xT
  q/k: qT = w.T @ xt (8 K-tiles, fp32 PSUM), bias folded into the
      PSUM->SBUF copy via DVE tensor_scalar_add, then RoPE (psgn matmul
      for rotate_half + cos/sin muls on DVE)
  V: VT[tok, 512 feats] = xt.T @ wv (8 K-tiles), PSUM->SBUF copy
      scatters into a [tok, kb, pair, h2, 80] resident layout whose
      65th column per head is pre-set to ones -> the PV matmul emits
      softmax row-sums for free
  attention per head-pair: S^T = KT.T @ QT (two heads row-tiled into
      one PE slot) -> exp on ACT (scale=1/8, bf16 out) -> causal via
      affine_select on diagonal blocks -> O[65, 2*512] += [V|1].T @ P^T
      (both heads accumulate into ONE 2-bank PSUM tile)
  normalize: one wide Ln + one wide Exp(-x) on [1, 1024] (both heads),
      one gpsimd partition_broadcast, DVE muls -> ot bf16
  proj: yT += pw.T @ ot (4 K-tiles per out tile), copies on DVE,
      output DMA on the ACT hardware queue (inputs use the SYNC queue)

Scheduling: engines execute their queues in order, and the PE p-state
only reaches full clock after ~3us of continuous execution, so the
emission order interleaves next-chunk qkv / previous-chunk projection
"filler" units between the score and PV matmuls of the attention inner
loop to keep the PE stream dense while ACT crunches the exps.  The VT
units of a chunk are emitted just-in-time ahead of the diagonal PV
blocks that consume them.
"""

import collections

import numpy as np
import ml_dtypes

import concourse.bass as bass  # noqa: F401
import concourse.tile as tile
from concourse import mybir, bacc
from concourse import bass_utils

f32 = mybir.dt.float32
bf16 = mybir.dt.bfloat16
AL = mybir.AluOpType
AF = mybir.ActivationFunctionType


class _Bacc(bacc.Bacc):
    """Pin activations to the table set holding both ln and exp so the
    per-pass Ln<->Exp pair doesn't thrash ACT_TABLE_LOADs."""

    def insert_act_table_loads(self):
        import bass_rust as _bass_rust
        from concourse.hw_specs import get_activation_tables

        has_activation = any(
            isinstance(i, mybir.InstActivation)
            for bb in self.main_func.blocks
            for i in bb.instructions
        )
        if not has_activation:
            return
        tables = [
            (k, v if k == "natural_log_exp_and_others" else set())
            for k, v in get_activation_tables(self.m.arch).items()
        ]
        _bass_rust.insert_act_table_loads(self, tables)


HIDDEN = 1024
HEADS = 16
HD = 64
ROPE_BASE = 10000.0
N_CORES = 8
H8 = 8            # heads per core
NPAIR = 4         # head pairs per core
FQ = NPAIR * 128  # 512 q (or k, or v) feature columns per core
QCH = 512         # token chunk = attention q granule
DT = HIDDEN // 128  # 8 k-tiles for the qkv GEMM


def build_program(T):
    """Per-core program: one batch of T tokens, 8 heads (4 pairs)."""
    assert T % QCH == 0
    NCH = T // QCH
    NKT = T // 128
    nc = _Bacc("TRN2", target_bir_lowering=False, debug=False,
               num_devices=N_CORES)

    xT = nc.dram_tensor("xT", [HIDDEN, T], bf16, kind="ExternalInput").ap()
    # q/k weights, ft-major: [128, ft(8) x dt(8) x 128]; ft = f*4+pair
    wqk = nc.dram_tensor("wqk", [128, 8 * DT * 128], bf16,
                         kind="ExternalInput").ap()
    # v weights, dt-major: [128, dt(8) x 512]
    wv = nc.dram_tensor("wv", [128, DT * FQ], bf16,
                        kind="ExternalInput").ap()
    bcol = nc.dram_tensor("bcol", [128, 8], f32, kind="ExternalInput").ap()
    psgn = nc.dram_tensor("psgn", [128, 128], bf16, kind="ExternalInput").ap()
    pw = nc.dram_tensor("pw", [128, NPAIR * HIDDEN], bf16,
                        kind="ExternalInput").ap()
    cos_t = nc.dram_tensor("cos_t", [128, T], bf16, kind="ExternalInput").ap()
    sin_t = nc.dram_tensor("sin_t", [128, T], bf16, kind="ExternalInput").ap()
    yT = nc.dram_tensor("yT", [HIDDEN, T], f32, kind="ExternalOutput").ap()

    with tile.TileContext(nc) as tc:
        with tc.tile_pool(name="const", bufs=1) as constp, \
             tc.tile_pool(name="resident", bufs=1) as resp, \
             tc.tile_pool(name="xt", bufs=16) as xtp, \
             tc.tile_pool(name="rope", bufs=4) as ropep, \
             tc.tile_pool(name="qt", bufs=8) as qtp, \
             tc.tile_pool(name="pt", bufs=5) as ptp, \
             tc.tile_pool(name="ot", bufs=16) as otp, \
             tc.tile_pool(name="ysb", bufs=3) as yp, \
             tc.tile_pool(name="small", bufs=2) as smp, \
             tc.tile_pool(name="ps_s", bufs=2, space="PSUM") as ps_s_p, \
             tc.tile_pool(name="ps_o", bufs=1, space="PSUM") as ps_o_p, \
             tc.tile_pool(name="ps_m", bufs=2, space="PSUM") as ps_m_p:

            # ---- constants / residents ----
            # Emission order of the input DMAs is their queue order; the
            # first qkv units need wqk[ft=0] (q, pair 0) and wqk[ft=4]
            # (k, pair 0) plus xt -- those go first on the SYNC queue.
            # Small constants stream in parallel on the ACT queue.
            wqk_sb = [None] * 8
            for ft in (0, 4):
                wqk_sb[ft] = constp.tile([128, DT * 128], bf16,
                                         name=f"wqk{ft}")
                nc.sync.dma_start(wqk_sb[ft][:],
                                  wqk[:, ft * DT * 128:(ft + 1) * DT * 128])
            bias_sb = constp.tile([128, 8], f32)
            nc.scalar.dma_start(bias_sb[:], bcol[:])
            psgn_sb = constp.tile([128, 128], bf16)
            nc.scalar.dma_start(psgn_sb[:], psgn[:])
            cos_sb = constp.tile([128, T], bf16)
            nc.scalar.dma_start(cos_sb[:], cos_t[:])
            sin_sb = constp.tile([128, T], bf16)
            nc.scalar.dma_start(sin_sb[:], sin_t[:])

            ST = {}  # (qc, pair) -> QT tile;  ("ot", qc, pair) -> ot tile

            def emit_xt_loads(qc):
                if ("xt", qc) in ST:
                    return
                t0 = qc * QCH
                xts = []
                for dt in range(DT):
                    t_ = xtp.tile([128, QCH], bf16, tag="xt", name=f"xt{dt}")
                    nc.sync.dma_start(t_[:],
                                      xT[dt * 128:(dt + 1) * 128, t0:t0 + QCH])
                    xts.append(t_)
                ST[("xt", qc)] = xts

            emit_xt_loads(0)

            wv_sb = constp.tile([128, DT * FQ], bf16, name="wv")
            nc.scalar.dma_start(wv_sb[:], wv[:])
            pw_sb = constp.tile([128, NPAIR * HIDDEN], bf16, name="pw")
            nc.scalar.dma_start(pw_sb[:], pw[:])
            for ft in (1, 5):
                wqk_sb[ft] = constp.tile([128, DT * 128], bf16,
                                         name=f"wqk{ft}")
                nc.sync.dma_start(wqk_sb[ft][:],
                                  wqk[:, ft * DT * 128:(ft + 1) * DT * 128])
            if T > QCH:
                emit_xt_loads(1)  # ahead of the late wqk slices on the queue
            for ft in (2, 6, 3, 7):
                wqk_sb[ft] = constp.tile([128, DT * 128], bf16,
                                         name=f"wqk{ft}")
                nc.sync.dma_start(wqk_sb[ft][:],
                                  wqk[:, ft * DT * 128:(ft + 1) * DT * 128])

            ones_c = constp.tile([128, 128], bf16)
            nc.gpsimd.memset(ones_c[:], 1.0)

            KT_res = [resp.tile([128, T], bf16, name=f"KT{p}")
                      for p in range(NPAIR)]
            # V resident, token-major: [tok, kb, pair, h2, 80] where the
            # 80-block is [v(64) | ones | pad(15)] -> the PV lhsT is a
            # contiguous [128, 65] slice whose 65th column emits the
            # softmax row-sum.
            VT_res = resp.tile([128, NKT * NPAIR * 2 * 80], bf16, name="VT")
            VT5 = VT_res[:].rearrange("t (kb p h c) -> t kb p h c",
                                      kb=NKT, p=NPAIR, h=2)
            nc.gpsimd.tensor_copy(
                VT5[:, :, :, :, 64],
                ones_c[:].rearrange("t (a b h) -> t a b h", a=NKT, b=NPAIR))

            def qk_units(qc):
                """Per-pair q/k emission units for chunk qc."""
                t0 = qc * QCH
                per_pair = [collections.deque() for _ in range(NPAIR)]
                for p in range(NPAIR):
                    units = per_pair[p]
                    for f in range(2):  # 0=q, 1=k
                        ft = f * 4 + p
                        cell = {}

                        def unit_a(ft=ft, cell=cell, qc=qc):
                            ps_f = ps_m_p.tile([128, QCH], f32, tag="m",
                                               name="ps_f")
                            cell["ps"] = ps_f
                            xts = ST[("xt", qc)]
                            for dt in range(4):
                                nc.tensor.matmul(
                                    ps_f[:],
                                    wqk_sb[ft][:, dt * 128:(dt + 1) * 128],
                                    xts[dt][:], start=(dt == 0), stop=False)

                        def unit_b(ft=ft, f=f, p=p, cell=cell, qc=qc, t0=t0):
                            ps_f = cell["ps"]
                            xts = ST[("xt", qc)]
                            for dt in range(4, DT):
                                nc.tensor.matmul(
                                    ps_f[:],
                                    wqk_sb[ft][:, dt * 128:(dt + 1) * 128],
                                    xts[dt][:], start=False, stop=(dt == 7))
                            raw = ropep.tile([128, QCH], bf16, tag="raw",
                                             name="raw")
                            nc.vector.tensor_scalar_add(
                                raw[:], ps_f[:], bias_sb[:, ft:ft + 1])
                            ps_rot = ps_m_p.tile([128, QCH], f32, tag="m",
                                                 name="ps_rot")
                            nc.tensor.matmul(ps_rot[:], psgn_sb[:], raw[:],
                                             start=True, stop=True)
                            t1 = ropep.tile([128, QCH], bf16, tag="t1",
                                            name="t1")
                            nc.vector.tensor_tensor(
                                t1[:], raw[:], cos_sb[:, t0:t0 + QCH],
                                AL.mult)
                            t2 = ropep.tile([128, QCH], bf16, tag="t2",
                                            name="t2")
                            nc.vector.tensor_tensor(
                                t2[:], ps_rot[:], sin_sb[:, t0:t0 + QCH],
                                AL.mult)
                            if f == 0:
                                qt_ = qtp.tile([128, QCH], bf16, tag="qt",
                                               name=f"QT{p}")
                                ST[(qc, p)] = qt_
                                dst = qt_[:]
                            else:
                                dst = KT_res[p][:, t0:t0 + QCH]
                            nc.vector.tensor_tensor(dst, t1[:], t2[:],
                                                    AL.add)

                        units.append(unit_a)
                        units.append(unit_b)
                return per_pair

            def vt_units(qc):
                """VT emission units for chunk qc: per 128-token block tb,
                VT[tok, feat] = xt.T @ wv accumulated over 8 d-tiles."""
                kt0 = qc * QCH // 128
                units = collections.deque()
                for tb in range(4):
                    cell = {}

                    def unit_a(tb=tb, cell=cell, qc=qc):
                        ps_v = ps_m_p.tile([128, FQ], f32, tag="m",
                                           name="ps_v")
                        cell["ps"] = ps_v
                        xts = ST[("xt", qc)]
                        for dt in range(4):
                            nc.tensor.matmul(
                                ps_v[:],
                                xts[dt][:, tb * 128:(tb + 1) * 128],
                                wv_sb[:, dt * FQ:(dt + 1) * FQ],
                                start=(dt == 0), stop=False)

                    def unit_b(tb=tb, cell=cell, qc=qc, kt0=kt0):
                        ps_v = cell["ps"]
                        xts = ST[("xt", qc)]
                        for dt in range(4, DT):
                            nc.tensor.matmul(
                                ps_v[:],
                                xts[dt][:, tb * 128:(tb + 1) * 128],
                                wv_sb[:, dt * FQ:(dt + 1) * FQ],
                                start=False, stop=(dt == 7))
                        nc.vector.tensor_copy(
                            VT5[:, kt0 + tb, :, :, 0:64],
                            ps_v[:].rearrange("t (p h c) -> t p h c",
                                              p=NPAIR, h=2))

                    units.append(unit_a)
                    units.append(unit_b)
                return units

            def proj_units(qc):
                """Emission units for chunk qc's projection (needs norm)."""
                t0 = qc * QCH
                units = collections.deque()
                ots = [ST[("ot", qc, p)] for p in range(NPAIR)]
                for oi in range(8):
                    def unit(oi=oi, ots=ots, t0=t0):
                        ps_y = ps_m_p.tile([128, QCH], f32, tag="m",
                                           name="ps_y")
                        for kt in range(NPAIR):
                            nc.tensor.matmul(
                                ps_y[:],
                                pw_sb[:, kt * HIDDEN + oi * 128:
                                      kt * HIDDEN + (oi + 1) * 128],
                                ots[kt][:], start=(kt == 0),
                                stop=(kt == NPAIR - 1))
                        ysb = yp.tile([128, QCH], f32, tag="y", name="ysb")
                        nc.vector.tensor_copy(ysb[:], ps_y[:])
                        nc.scalar.dma_start(
                            yT[oi * 128:(oi + 1) * 128, t0:t0 + QCH], ysb[:])
                    units.append(unit)
                return units

            def attention_pass(qc, p, fillers):
                """One head-pair's attention for chunk qc.  `fillers` is a
                list of deques of emission units, pulled between the score
                and PV matmuls to keep the PE stream dense."""
                Q0 = qc * QCH
                nkb = (Q0 + QCH) // 128

                def pull(n=1):
                    for _ in range(n):
                        for fl in fillers:
                            if fl:
                                fl.popleft()()
                                break

                # both heads' O (and their ones-column row sums at
                # partition 64) accumulate into ONE 2-bank PSUM tile:
                # O[:, h2*QCH + q]
                O = ps_o_p.tile([65, 2 * QCH], f32, tag="o", name="O")
                for kb in range(nkb):
                    qs = max(0, 128 * kb - Q0)
                    ps_sc = ps_s_p.tile([128, 2 * QCH], f32, tag="s",
                                        name="ps_sc")
                    for h2 in range(2):
                        hp = slice(64 * h2, 64 * h2 + 64)
                        nc.tensor.matmul(
                            ps_sc[:, QCH * h2 + qs:QCH * h2 + QCH],
                            KT_res[p][hp, kb * 128:(kb + 1) * 128],
                            ST[(qc, p)][hp, qs:QCH],
                            start=True, stop=True)
                    pt = ptp.tile([128, 2 * QCH], bf16, tag="pt", name="pt")
                    sc4 = ps_sc[:].rearrange("p (h q) -> p h q", h=2)
                    pt4 = pt[:].rearrange("p (h q) -> p h q", h=2)
                    nc.scalar.activation(pt4[:, :, qs:QCH],
                                         sc4[:, :, qs:QCH],
                                         AF.Exp, bias=0.0, scale=0.125)
                    if 128 * kb >= Q0:
                        ds = 128 * kb - Q0
                        for h2 in range(2):
                            nc.gpsimd.affine_select(
                                out=pt[:, QCH * h2 + ds:QCH * h2 + ds + 128],
                                in_=pt[:, QCH * h2 + ds:QCH * h2 + ds + 128],
                                pattern=[[1, 128]], compare_op=AL.is_ge,
                                fill=0.0, base=0, channel_multiplier=-1)
                    pull(3 if kb == 0 else 1)
                    for h2 in range(2):
                        nc.tensor.matmul(
                            O[:, QCH * h2 + qs:QCH * h2 + QCH],
                            VT5[:, kb, p, h2, 0:65],
                            pt[:, QCH * h2 + qs:QCH * h2 + QCH],
                            start=(kb == 0), stop=(kb == nkb - 1))
                # normalize: 1/rowsum via exp(-ln(rowsum)), one wide pass
                # over both heads' row-sum rows (partition 64 of O).
                # Pull extra filler first: the ot muls sit at the head of
                # the strict-FIFO DVE queue while they wait on ln/exp +
                # broadcast, so any DVE op emitted after them (next
                # pair's rope) would stall the PE transitively.
                pull(4)
                ot_ = otp.tile([128, QCH], bf16, tag="ot", name=f"ot{p}")
                ST[("ot", qc, p)] = ot_
                lnv = smp.tile([1, 2 * QCH], f32, tag="ln", name="lnv")
                nc.scalar.activation(lnv[:], O[64:65, :], AF.Ln)
                rs = smp.tile([1, 2 * QCH], bf16, tag="rs", name="rs")
                nc.scalar.activation(rs[:], lnv[:], AF.Exp,
                                     bias=0.0, scale=-1.0)
                rsb = smp.tile([64, 2 * QCH], bf16, tag="rsb", name="rsb")
                nc.gpsimd.partition_broadcast(rsb[:], rs[:])
                for h2 in range(2):
                    nc.vector.tensor_tensor(
                        ot_[64 * h2:64 * h2 + 64, :],
                        O[0:64, QCH * h2:QCH * h2 + QCH],
                        rsb[:, QCH * h2:QCH * h2 + QCH], AL.mult)

            # ---------------- main schedule ----------------
            # attention pass (qc, p) pulls, in priority order: this
            # chunk's remaining VT units (just-in-time for the diagonal
            # PV blocks), the same chunk's next-pair q/k units, then (in
            # the last pass) the next chunk's pair-0 units, then the
            # previous chunk's projection.
            U = qk_units(0)
            V = vt_units(0)
            # bootstrap: pair-0 q/k plus the first VT block before the
            # first attention pass
            for u in U[0]:
                u()
            U[0].clear()
            for _ in range(2):
                V.popleft()()
            fp = collections.deque()
            for qc in range(NCH):
                if qc + 1 < NCH:
                    emit_xt_loads(qc + 1)
                    Un = qk_units(qc + 1)
                    Vn = vt_units(qc + 1)
                else:
                    Un = [collections.deque() for _ in range(NPAIR)]
                    Vn = collections.deque()
                for p in range(NPAIR):
                    for u in U[p]:  # force-drain this pair's leftovers
                        u()
                    U[p].clear()
                    if p == 1:
                        while V:  # VT must be emitted before pass 1's
                            V.popleft()()  # diagonal PVs at the latest
                    if p + 1 < NPAIR:
                        fillers = [V, U[p + 1], fp]
                    else:
                        fillers = [V, Un[0], Vn, fp]
                    attention_pass(qc, p, fillers)
                U = Un
                # next chunk's VT: first block force-emitted so pass
                # (qc+1, 0)'s kb0 PV never waits on emission order
                if Vn:
                    for _ in range(2):
                        Vn.popleft()()
                V = Vn
                # proj units are NOT force-drained per chunk: they stay
                # in fp as lowest-priority filler so PE work is left in
                # reserve for the filler-starved final chunk.
                fp.extend(proj_units(qc))
            while fp:
                fp.popleft()()
    nc.compile()
    return nc


# ---------------------------------------------------------------- host side

def _rope_tables(T):
    inv_freq = 1.0 / (ROPE_BASE ** (np.arange(0, HD, 2, dtype=np.float64)
                                    / HD))
    pos = np.arange(T, dtype=np.float64)
    ang = np.outer(pos, inv_freq)          # [T, 32]
    cos = np.cos(ang)
    sin = np.sin(ang)
    jm = (np.arange(128) % 64) % 32
    cos_t = np.ascontiguousarray(cos[:, jm].T).astype(ml_dtypes.bfloat16)
    sin_t = np.ascontiguousarray(sin[:, jm].T).astype(ml_dtypes.bfloat16)
    return cos_t, sin_t


def _psgn():
    p = np.zeros((HD, HD), np.float32)
    for i in range(32):
        p[i + 32, i] = -1.0   # out dim i (<32) = -in dim i+32
        p[i, i + 32] = 1.0    # out dim i+32   = +in dim i
    pf = np.zeros((128, 128), np.float32)
    pf[0:64, 0:64] = p        # head-even block
    pf[64:128, 64:128] = p    # head-odd block
    return np.ascontiguousarray(pf).astype(ml_dtypes.bfloat16)


def make_core_inputs(x, qkv_w, qkv_b, proj_w, B, T):
    x = np.asarray(x, dtype=np.float32)
    qkv_w = np.asarray(qkv_w, dtype=np.float32)
    qkv_b = np.asarray(qkv_b, dtype=np.float32)
    proj_w = np.asarray(proj_w, dtype=np.float32)
    cos_t, sin_t = _rope_tables(T)
    psgn = _psgn()
    xTs = [np.ascontiguousarray(x[b].T).astype(ml_dtypes.bfloat16)
           for b in range(B)]
    in_maps = []
    for c in range(N_CORES):
        b, g = divmod(c, 2)
        col = FQ * g
        # q/k weights, ft-major: ft = f*4 + pair
        blocks = []
        for f in range(2):
            for p in range(NPAIR):
                sl = qkv_w[:, f * HIDDEN + col + 128 * p:
                           f * HIDDEN + col + 128 * (p + 1)]      # [1024,128]
                blocks.append(sl.reshape(DT, 128, 128)
                              .transpose(1, 0, 2).reshape(128, DT * 128))
        wqk_c = np.ascontiguousarray(np.concatenate(blocks, axis=1)) \
            .astype(ml_dtypes.bfloat16)                           # [128,8192]
        # v weights, dt-major
        slv = qkv_w[:, 2 * HIDDEN + col:2 * HIDDEN + col + FQ]    # [1024,512]
        wv_c = np.ascontiguousarray(
            slv.reshape(DT, 128, FQ).transpose(1, 0, 2)
            .reshape(128, DT * FQ)).astype(ml_dtypes.bfloat16)
        bc = np.zeros((128, 8), np.float32)
        for f in range(2):
            for p in range(NPAIR):
                bc[:, f * 4 + p] = qkv_b[f * HIDDEN + col + 128 * p:
                                         f * HIDDEN + col + 128 * (p + 1)]
        pwc = np.ascontiguousarray(
            proj_w[col:col + FQ, :].reshape(NPAIR, 128, HIDDEN)
            .transpose(1, 0, 2).reshape(128, NPAIR * HIDDEN)
        ).astype(ml_dtypes.bfloat16)
        in_maps.append({
            "xT": xTs[b], "wqk": wqk_c, "wv": wv_c,
            "bcol": np.ascontiguousarray(bc),
            "psgn": psgn, "pw": pwc, "cos_t": cos_t, "sin_t": sin_t,
        })
    return in_maps


_PROGRAM_CACHE = {}


def _get_program(T):
    if T not in _PROGRAM_CACHE:
        _PROGRAM_CACHE[T] = build_program(T)
    return _PROGRAM_CACHE[T]


def run(x, qkv_w, qkv_b, proj_w, proj_b, NB, T, trace=False):
    nc = _get_program(T)
    in_maps = make_core_inputs(x, qkv_w, qkv_b, proj_w, NB, T)
    res = bass_utils.run_bass_kernel_spmd(
        nc, in_maps, core_ids=list(range(N_CORES)), trace=trace)
    # v-bias passes through attention unchanged (softmax rows sum to 1),
    # so fold it into an effective proj bias on the host -- exact.
    qkv_b = np.asarray(qkv_b, dtype=np.float32)
    pb_eff = (np.asarray(proj_b, dtype=np.float32)
              + qkv_b[2 * HIDDEN:] @ np.asarray(proj_w, dtype=np.float32))
    out = np.empty((NB, T, HIDDEN), np.float32)
    for b in range(NB):
        acc = res.results[2 * b]["yT"].astype(np.float32) \
            + res.results[2 * b + 1]["yT"]
        out[b] = acc.T
    out += pb_eff[None, None, :]
    return out, res


def kernel(x, qkv_w, qkv_b, proj_w, proj_b):
    x = np.asarray(x)
    B, L, D = x.shape
    assert D == HIDDEN and B % 2 == 0 and N_CORES == 2 * B
    out, _ = run(x, np.asarray(qkv_w), np.asarray(qkv_b),
                 np.asarray(proj_w), np.asarray(proj_b), NB=B, T=L)
    return out.astype(np.float32)
